# revision 23
# baseline (speedup 1.0000x reference)
"""Additive (Bahdanau) attention kernel for 8 TRN2 NeuronCores.

Problem (full shapes): H=1024, B=64, S=2048
    enc = transpose(encoder_states, (1,0,2))            # (B,S,H)
    proj_prev = decoder_prev_state @ Wp.T               # (B,H)
    proj_enc  = enc @ We.T                              # (B,S,H)
    scores    = einsum('bsh,h->bs', tanh(pp+pe), v)     # (B,S)
    attn      = softmax(where(mask==0, -inf, scores))
    out       = einsum('bsh,bs->bh', enc, attn)         # (B,H)

Sharding: data-parallel over batch. Each of the 8 cores handles 8 batch
rows; the three small weights are replicated. No collectives.

Per-core dataflow (all matmuls bf16 on the PE, f32 PSUM accumulate):
  - cast-load enc tile (512 s-rows of one b) natural layout -> bf16  [s,  h]
  - one SBUF->SBUF DMA-transpose (xbar)                    -> bf16  [h,  s]
  - proj^T[m,:] = sum_k WeT[k,m].T @ encT[k]  (8x8 matmuls, N=512)
  - ScalarE: tanh(psum + qprojT[:,b]) fused via activation bias
  - score   = sum_m vT[m].T @ tanh[m]         (matvec matmuls, M=1)
  - p = exp(score) * maskf  (no max subtraction needed: |score| <= ||v||*32,
    exp stays finite in f32); denominator via fused tensor_tensor_reduce
  - p transposed to partitions via a [16,128] xbar DMA (rows 1..15 padding)
  - context numerator = sum_s p[s] * enc[s,:] as matmuls with p stationary
  - final: out = num / den
"""

import numpy as np

H = 1024
B = 64
S = 2048
NCORES = 8
BL = B // NCORES  # 8 batch rows per core
P = 128
ST = 512          # s-tile
NST = S // ST     # 4
C4 = ST // P      # 4 partition-chunks per s-tile
KC = H // P       # 8 h-chunks

_CACHE = {}


def _build_bass():
    import concourse.bass as bass
    import concourse.mybir as mybir
    import concourse.tile as tile
    import concourse.tile as _tile
    import concourse.tile_sem_assignment as _tsa

    # Pin every DMA to a single completion-semaphore lane per DGE type.  All
    # HWDGE DMAs here issue from one FIFO ring (nc.sync), so same-lane deps
    # ride program order with no wait commands.  This matters because the
    # DMA-transpose (XPOSE) instruction carries at most ONE sync-wait slot;
    # with the default 8-lane round-robin, a transpose waiting on its
    # producer DMA plus a slot-WAR lands 2-3 waits and walrus rejects it.
    _tsa.NUM_HWDGE_SEMS = 1
    _tsa.NUM_SWDGE_GLOBAL_SEMS = 1

    fp32 = mybir.dt.float32
    bf16 = mybir.dt.bfloat16
    i32 = mybir.dt.int32
    Tanh = mybir.ActivationFunctionType.Tanh
    Exp = mybir.ActivationFunctionType.Exp
    mult = mybir.AluOpType.mult
    add = mybir.AluOpType.add

    nc = bass.Bass()

    shim_state = {}

    def shimmed_transpose(out_full, in_full, out_corner, in_corner):
        """Every HWDGE DMA pseudo-instruction carries at most ONE HW
        sync-wait slot, but a DMA-transpose naturally needs waits on up to
        three sem domains (compute producer RAW + dest-slot WAR on PE + WAW
        on the HWDGE lane).  Since all HWDGE DMAs here share one FIFO ring
        and one completion-sem lane, earlier DMAs' waits advance the lane's
        observed clock — so we chain tiny single-wait carrier copies:
          d1 reads the source corner   -> carries the producer (DVE) wait
          d2 writes the dest corner    -> carries the dest-slot WAR (PE)
          XPOSE                        -> carries only d2's completion
        A nosync edge d1->d2 pins scheduling order (the ring is FIFO, so
        d1's wait has executed before d2 issues)."""
        n = in_corner.free_size()
        scr = shim_state["pool"].tile([1, 128], in_full.dtype, tag="scr", name="scr")
        d1 = nc.sync.dma_start(out=scr[0:1, 0:n], in_=in_corner)
        d2 = nc.sync.dma_start(out=out_corner, in_=in_corner)
        _tile.add_dep_helper(d2.ins, d1.ins, sync=False, reason="shim chain d1->d2")
        nc.sync.dma_start(out=out_full, in_=in_full, transpose=True)

    enc = nc.dram_tensor("encoder_states", [S, BL, H], fp32, kind="ExternalInput")
    dec = nc.dram_tensor("decoder_prev_state", [BL, H], fp32, kind="ExternalInput")
    msk = nc.dram_tensor("mask", [BL, S], i32, kind="ExternalInput")
    Wp = nc.dram_tensor("Wp", [H, H], fp32, kind="ExternalInput")
    We = nc.dram_tensor("We", [H, H], fp32, kind="ExternalInput")
    v = nc.dram_tensor("v", [H], fp32, kind="ExternalInput")
    out = nc.dram_tensor("out", [BL, H], fp32, kind="ExternalOutput")

    with tile.TileContext(nc) as tc:
        with (
            tc.tile_pool(name="consts", bufs=1) as consts,
            tc.tile_pool(name="wstage", bufs=2) as wstage,
            tc.tile_pool(name="xf", bufs=2) as xf_pool,
            tc.tile_pool(name="xa", bufs=2) as xa_pool,
            tc.tile_pool(name="xt", bufs=2) as xt_pool,
            tc.tile_pool(name="th", bufs=3) as th_pool,
            tc.tile_pool(name="sm", bufs=4) as sm,
            tc.tile_pool(name="pp", bufs=2) as pp_pool,
            tc.tile_pool(name="pj", bufs=2, space="PSUM") as psum_pj,
            tc.tile_pool(name="ps", bufs=2, space="PSUM") as psum_s,
            tc.tile_pool(name="pn", bufs=2, space="PSUM") as psum_n,
            tc.tile_pool(name="scr", bufs=4) as scr_pool,
        ):
            shim_state["pool"] = scr_pool
            # ---------- setup: transposed weights (one-time) ----------
            # All HBM loads use HWDGE (nc.sync) and cast f32->bf16 on the DVE,
            # so every XPOSE dependency is either the single HWDGE lane
            # (program order) or a DVE sem already waited on by its shim.
            # WT[p, mc, k, f] = W[mc*128+f, k*128+p]  (bf16)
            WeT = consts.tile([P, KC, KC, P], bf16, tag="WeT", name="WeT")
            WpT = consts.tile([P, KC, KC, P], bf16, tag="WpT", name="WpT")
            for W_hdl, WT in ((We, WeT), (Wp, WpT)):
                for mc in range(KC):
                    wf32 = wstage.tile([P, H], fp32, tag="wf32", name="wf32")
                    nc.sync.dma_start(out=wf32[:], in_=W_hdl[mc * P:(mc + 1) * P, :])
                    stg = wstage.tile([P, H], bf16, tag="wstg", name="wstg")
                    nc.vector.tensor_copy(out=stg[:], in_=wf32[:])
                    shimmed_transpose(
                        WT[:, mc, :, :], stg[:],
                        WT[0:1, mc, 0:1, 0:16], stg[0:1, 0:16],
                    )

            # decT[p, k, f] = dec[f, k*128+p] for f<8 (rows 8..15 zero pad)
            d_f32 = consts.tile([BL, H], fp32, tag="d_f32", name="d_f32")
            nc.sync.dma_start(out=d_f32[:], in_=dec[:, :])
            dec_bf = consts.tile([16, H], bf16, tag="dec_bf", name="dec_bf")
            nc.vector.memset(dec_bf[:], 0.0)
            nc.vector.tensor_copy(out=dec_bf[0:BL, :], in_=d_f32[:])
            decTt = consts.tile([P, KC, 16], bf16, tag="decTt", name="decTt")
            shimmed_transpose(decTt[:], dec_bf[:], decTt[0:1, 0:1, 0:16], dec_bf[0:1, 0:16])

            # vT[p, k, 0] = v[k*128+p]
            v_f32 = consts.tile([1, H], fp32, tag="v_f32", name="v_f32")
            nc.sync.dma_start(out=v_f32[:], in_=v[:])
            v_sb = consts.tile([16, H], bf16, tag="v_sb", name="v_sb")
            nc.vector.memset(v_sb[:], 0.0)
            nc.vector.tensor_copy(out=v_sb[0:1, :], in_=v_f32[:])
            vT = consts.tile([P, KC, 16], bf16, tag="vT", name="vT")
            shimmed_transpose(vT[:], v_sb[:], vT[0:1, 0:1, 0:16], v_sb[0:1, 0:16])

            # qprojT[p, mc, b] = (Wp @ dec[b])[mc*128+p]
            qprojT = consts.tile([P, KC, BL], fp32, tag="qprojT", name="qprojT")
            for mc in range(KC):
                pq = psum_pj.tile([P, 512], fp32, tag="pj", name="pq")
                for k in range(KC):
                    nc.tensor.matmul(
                        pq[:, 0:BL],
                        lhsT=WpT[:, mc, k, :],
                        rhs=decTt[:, k, 0:BL],
                        start=(k == 0),
                        stop=(k == KC - 1),
                    )
                nc.vector.tensor_copy(out=qprojT[:, mc, :], in_=pq[:, 0:BL])

            # ---------- main loop ----------
            for b in range(BL):
                nlo = psum_n.tile([1, 512], fp32, tag="nlo", name="nlo")
                nhi = psum_n.tile([1, 512], fp32, tag="nhi", name="nhi")
                den = sm.tile([1, NST], fp32, tag="den", name="den")
                for st in range(NST):
                    # natural-layout load: xa[p, c, h] = enc[st*512+c*128+p, b, h]
                    xf = xf_pool.tile([P, C4, H], fp32, tag="xf", name="xf")
                    src = enc[st * ST:(st + 1) * ST, b, :].rearrange(
                        "(c p) h -> p c h", p=P
                    )
                    nc.sync.dma_start(out=xf[:], in_=src)
                    xa = xa_pool.tile([P, C4, H], bf16, tag="xa", name="xa")
                    nc.vector.tensor_copy(out=xa[:], in_=xf[:])

                    # xbar transpose: xt[p, c, k, f] = xa[f, c, k*128+p]
                    # shim dest corner spans all k so it carries the full WAR
                    xt = xt_pool.tile([P, C4, KC, P], bf16, tag="xt", name="xt")
                    shimmed_transpose(
                        xt[:], xa[:].rearrange("p c h -> p (c h)"),
                        xt[0:1, 0:1, :, 0:16], xa[0:1, 0:1, 0:128],
                    )

                    # proj^T + fused tanh(x + qproj_b)
                    th_tiles = []
                    for mc in range(KC):
                        pj = psum_pj.tile([P, 512], fp32, tag="pj", name="pj")
                        for k in range(KC):
                            nc.tensor.matmul(
                                pj[:],
                                lhsT=WeT[:, mc, k, :],
                                rhs=xt[:, :, k, :],
                                start=(k == 0),
                                stop=(k == KC - 1),
                            )
                        th = th_pool.tile([P, ST], bf16, tag="th", name="th")
                        nc.scalar.activation(
                            out=th[:],
                            in_=pj[:],
                            func=Tanh,
                            bias=qprojT[:, mc, b:b + 1],
                            scale=1.0,
                        )
                        th_tiles.append(th)

                    # scores[0, s'] = sum_h v[h] * tanh[h, s']
                    ps = psum_s.tile([1, 512], fp32, tag="ps", name="ps")
                    for mc in range(KC):
                        nc.tensor.matmul(
                            ps[:],
                            lhsT=vT[:, mc, 0:1],
                            rhs=th_tiles[mc][:],
                            start=(mc == 0),
                            stop=(mc == KC - 1),
                        )

                    ex = sm.tile([1, ST], fp32, tag="ex", name="ex")
                    nc.scalar.activation(out=ex[:], in_=ps[:], func=Exp)

                    mi = sm.tile([1, ST], i32, tag="mi", name="mi")
                    nc.sync.dma_start(out=mi[:], in_=msk[b:b + 1, st * ST:(st + 1) * ST])
                    mf = sm.tile([1, ST], fp32, tag="mf", name="mf")
                    nc.vector.tensor_copy(out=mf[:], in_=mi[:])

                    # p = ex * maskf (bf16, row 0); den[st] = sum_s p
                    pst = pp_pool.tile([16, ST], bf16, tag="pst", name="pst")
                    nc.vector.memset(pst[:], 0.0)
                    nc.vector.tensor_tensor(
                        out=pst[0:1, :], in0=ex[:], in1=mf[:], op=mult
                    )
                    nc.vector.reduce_sum(
                        out=den[:, st:st + 1],
                        in_=pst[0:1, :],
                        axis=mybir.AxisListType.X,
                    )

                    # pT[p, c, 0] = p[c*128+p]
                    pT = pp_pool.tile([P, C4, 16], bf16, tag="pT", name="pT")
                    pscr = scr_pool.tile([1, 128], bf16, tag="scr", name="pscr")
                    pd1 = nc.sync.dma_start(out=pscr[0:1, 0:64], in_=pst[0:1, 0:64])
                    pd2 = nc.sync.dma_start(out=pT[0:1, :, 0:16], in_=pst[0:1, 0:64])
                    _tile.add_dep_helper(
                        pd2.ins, pd1.ins, sync=False, reason="pT shim chain"
                    )
                    for c in range(C4):
                        nc.sync.dma_start(
                            out=pT[:, c, :],
                            in_=pst[:, c * P:(c + 1) * P],
                            transpose=True,
                        )

                    # numerator accumulation: num[h] += sum_s p[s]*enc[s,b,h]
                    for c in range(C4):
                        nc.tensor.matmul(
                            nlo[:],
                            lhsT=pT[:, c, 0:1],
                            rhs=xa[:, c, 0:512],
                            start=(st == 0 and c == 0),
                            stop=(st == NST - 1 and c == C4 - 1),
                        )
                        nc.tensor.matmul(
                            nhi[:],
                            lhsT=pT[:, c, 0:1],
                            rhs=xa[:, c, 512:1024],
                            start=(st == 0 and c == 0),
                            stop=(st == NST - 1 and c == C4 - 1),
                        )

                # finalize: out[b] = num / den
                import concourse.mybir as _mybir
                dtot = sm.tile([1, 1], fp32, tag="dtot", name="dtot")
                nc.vector.reduce_sum(out=dtot[:], in_=den[:], axis=_mybir.AxisListType.X)
                inv = sm.tile([1, 1], fp32, tag="inv", name="inv")
                nc.vector.reciprocal(out=inv[:], in_=dtot[:])
                cx = sm.tile([1, H], fp32, tag="cx", name="cx")
                nc.vector.tensor_scalar_mul(cx[:, 0:512], nlo[:], inv[:])
                nc.vector.tensor_scalar_mul(cx[:, 512:1024], nhi[:], inv[:])
                nc.sync.dma_start(out=out[b:b + 1, :], in_=cx[:])

    _legalize_dma_waits(nc)
    return nc


def _legalize_dma_waits(nc):
    """This container's walrus enforces per-instruction sync budgets the Tile
    pipeline does not respect: most ISA encodings carry at most ONE sync-wait
    slot (EventSemaphore holds two), and the 64-byte-padded
    EVENT_SEMAPHORE_RANGE_CLEAR InstISA is rejected outright.  Legalize after
    Tile: move excess waits onto standalone EventSemaphore instructions
    inserted just before the instruction on the same engine stream (the
    sequencer executes them in order, so the instruction still issues only
    after all its waits are satisfied), and replace the range-clear with
    per-semaphore EventSemaphore writes of 0."""
    import concourse.mybir as mybir
    import bass_rust

    nev = [0]

    def mkev(engine, waits, updates=()):
        ev = mybir.InstEventSemaphore(name=f"evw-{nev[0]}", ins=[], outs=[])
        nev[0] += 1
        ev.engine = engine
        ev.sync_info = bass_rust.SyncInfo(
            on_wait=list(waits), on_update=list(updates)
        )
        return ev

    for blk in nc.m.functions[0].blocks:
        insts = blk.instructions
        new = []
        for inst in insts:
            t = type(inst).__name__
            si = getattr(inst, "sync_info", None)
            cap = 2 if t == "InstEventSemaphore" else 1
            if si is not None and len(si.on_wait) > cap:
                waits = list(si.on_wait)
                extra, keep = waits[:-cap], waits[-cap:]
                for j in range(0, len(extra), 2):
                    new.append(mkev(inst.engine, extra[j:j + 2]))
                inst.sync_info = bass_rust.SyncInfo(
                    on_wait=keep, on_update=list(si.on_update)
                )
            if t == "InstISA" and getattr(inst, "op_name", "") == (
                "EVENT_SEMAPHORE_RANGE_CLEAR"
            ):
                ib = list(inst.instr)
                lo, hi = ib[13], ib[14]
                del lo, hi  # single-run NEFF: sems needn't be recycled
                continue
            new.append(inst)
        try:
            blk.instructions = new
        except Exception:
            insts.clear()
            insts.extend(new)


def _get_nc():
    if "nc" not in _CACHE:
        _CACHE["nc"] = _build_bass()
    return _CACHE["nc"]


def _make_in_maps(inputs):
    enc = np.ascontiguousarray(np.asarray(inputs["encoder_states"], dtype=np.float32))
    dec = np.ascontiguousarray(np.asarray(inputs["decoder_prev_state"], dtype=np.float32))
    msk = np.ascontiguousarray(np.asarray(inputs["mask"], dtype=np.int32))
    Wp = np.ascontiguousarray(np.asarray(inputs["Wp"], dtype=np.float32))
    We = np.ascontiguousarray(np.asarray(inputs["We"], dtype=np.float32))
    v = np.ascontiguousarray(np.asarray(inputs["v"], dtype=np.float32))

    in_maps = []
    for i in range(NCORES):
        sl = slice(i * BL, (i + 1) * BL)
        in_maps.append(
            {
                "encoder_states": np.ascontiguousarray(enc[:, sl, :]),
                "decoder_prev_state": np.ascontiguousarray(dec[sl, :]),
                "mask": np.ascontiguousarray(msk[sl, :]),
                "Wp": Wp,
                "We": We,
                "v": v,
            }
        )
    return in_maps


def kernel_profiled(trace=False, **inputs):
    """Run on 8 cores; returns (full_output, BassKernelResults)."""
    from concourse.bass_utils import run_bass_kernel_spmd

    nc = _get_nc()
    in_maps = _make_in_maps(inputs)
    res = run_bass_kernel_spmd(nc, in_maps, core_ids=list(range(NCORES)), trace=trace)
    out = np.concatenate([r["out"] for r in res.results], axis=0)
    return out.astype(np.float32), res


def kernel(**inputs):
    out, _ = kernel_profiled(trace=False, **inputs)
    return out
